# revision 1
# baseline (speedup 1.0000x reference)
"""Trainium2 Bass kernel for nn_Net_88381837017215 (2-layer GCN message passing).

  h = relu(A @ (features @ W1)); o = softmax(relu(A @ (h @ W2)))

Strategy (8 NeuronCores, SPMD, 3 launches with host gather between):
- Nodes row-sharded: core c owns rows [c*12500,(c+1)*12500), padded to 12544
  (98 windows x 128). Global padded tables: 100352 rows.
- Launch A: x1 = features @ W1 per shard (fp32 PSUM, fp16 out).
  Host concatenates the 8 shards into the full x1 table.
- Launch B: spmm1 + relu + dense2. Edges grouped by owner row-window (128 dst
  nodes) and source-chunk (4 chunks of 25088 table rows so gather indices fit
  int16); each (window,chunk) padded to quota[chunk] tiles of 128 edges. Per
  super-block of SB=7 windows, one bulk dma_gather per chunk fetches the edge
  source rows (fp16, 256B each). Segment-sum via one-hot matmuls:
  S[e,n] = val[e] * (row_local[e]==n) built fp16 with block DVE ops; PE
  accumulates msgs.T @ S into PSUM (output lands transposed, feeding h @ W2
  directly without an explicit transpose). Host concatenates x2 shards.
- Launch C: spmm2 (S.T @ msgs) + relu + on-chip softmax.

kernel(**inputs) takes FULL inputs, shards on host, runs on cores 0-7 via
run_bass_kernel_spmd, returns the FULL [100000, 64] float32 output.
"""
import os
import sys

for _p in ("/opt/trn_rl_repo", "/root/.axon_site/_ro/trn_rl_repo"):
    if os.path.isdir(_p):
        sys.path.insert(0, _p)
        break

import numpy as np

NCORES = 8
N = 100000
P = 128
NSHARD = N // NCORES            # 12500
NWIN = (NSHARD + P - 1) // P    # 98
NPADC = NWIN * P                # 12544
NTOT = NCORES * NPADC           # 100352
NCHUNK = 4
CHROWS = NTOT // NCHUNK         # 25088
SB = 7
NSB = NWIN // SB                # 14
HID, OUT, IN_F = 128, 64, 256


# ---------------------------------------------------------------- host side

def _preprocess(edge_row, edge_col, edge_val):
    core = edge_row // NSHARD
    rlc = edge_row % NSHARD
    win = rlc // P
    row_in_win = rlc % P
    colp = (edge_col // NSHARD) * NPADC + (edge_col % NSHARD)
    chunk = colp // CHROWS
    idx16 = (colp % CHROWS).astype(np.int32)

    key = (core * NWIN + win) * NCHUNK + chunk
    counts = np.bincount(key, minlength=NCORES * NWIN * NCHUNK)
    counts = counts.reshape(NCORES, NWIN, NCHUNK)
    quota = np.ceil(counts.max(axis=(0, 1)) / P).astype(np.int64)
    T = int(quota.sum())

    order = np.argsort(key, kind="stable")
    s_riw = row_in_win[order]
    s_idx = idx16[order]
    s_val = edge_val[order]

    starts = np.zeros(NCORES * NWIN * NCHUNK + 1, np.int64)
    np.cumsum(counts.reshape(-1), out=starts[1:])
    off = np.concatenate([[0], np.cumsum(quota)])
    per_core = []
    for c in range(NCORES):
        idx_arr = np.zeros((NWIN, T, P), np.int16)
        rl_arr = np.zeros((NWIN, T, P), np.float16)
        val_arr = np.zeros((NWIN, T, P), np.float16)
        for w in range(NWIN):
            g0 = (c * NWIN + w) * NCHUNK
            for k in range(NCHUNK):
                a, b = starts[g0 + k], starts[g0 + k + 1]
                n = b - a
                base = int(off[k]) * P
                idx_arr[w].reshape(-1)[base:base + n] = s_idx[a:b]
                rl_arr[w].reshape(-1)[base:base + n] = s_riw[a:b]
                val_arr[w].reshape(-1)[base:base + n] = s_val[a:b]
        per_core.append((idx_arr, rl_arr, val_arr))
    return quota, per_core


def _build_edge_inputs(edge_row, edge_col, edge_val):
    quota, per_core = _preprocess(edge_row, edge_col, edge_val)
    T = int(quota.sum())
    edge_maps = []
    for c in range(NCORES):
        idx_arr, rl_arr, val_arr = per_core[c]
        calls = []
        for sb in range(NSB):
            o = 0
            for k in range(NCHUNK):
                q = int(quota[k])
                blk = idx_arr[sb * SB:(sb + 1) * SB, o:o + q, :]
                calls.append(blk.reshape(-1).reshape(-1, 16).T)
                o += q
        idx_all = np.tile(np.concatenate(calls, axis=1), (8, 1))
        rl_all = np.ascontiguousarray(
            rl_arr.transpose(2, 0, 1).reshape(P, NWIN * T))
        val_all = np.ascontiguousarray(
            val_arr.transpose(2, 0, 1).reshape(P, NWIN * T))
        edge_maps.append({
            "idx_all": np.ascontiguousarray(idx_all, dtype=np.int16),
            "rl_all": rl_all,
            "val_all": val_all,
        })
    return quota, edge_maps


# ------------------------------------------------------------- bass programs

_CACHE = {}


def _bass_mods():
    import concourse.bacc as bacc
    import concourse.tile as tile
    from concourse import mybir
    return bacc, tile, mybir


def _build_prog_a():
    """x1_shard[NPADC, HID] (fp16) = featT.T @ W1 (fp32 accum)."""
    bacc, tile, mybir = _bass_mods()
    f32, f16 = mybir.dt.float32, mybir.dt.float16
    AF = mybir.ActivationFunctionType

    nc = bacc.Bacc("TRN2", target_bir_lowering=False, debug=False,
                   num_devices=NCORES)
    featT = nc.dram_tensor("featT", [IN_F, NPADC], f32, kind="ExternalInput")
    W1 = nc.dram_tensor("W1", [IN_F, HID], f32, kind="ExternalInput")
    x1 = nc.dram_tensor("x1", [NPADC, HID], f16, kind="ExternalOutput")

    with tile.TileContext(nc, num_cores=NCORES) as tc:
        with tc.tile_pool(name="const", bufs=1) as cpool, \
             tc.tile_pool(name="io", bufs=4) as iopool, \
             tc.tile_pool(name="ps", bufs=4, space="PSUM") as pspool:
            W1a = cpool.tile([P, HID], f32, tag="W1a")
            nc.sync.dma_start(out=W1a[:], in_=W1[0:P, :])
            W1b = cpool.tile([P, HID], f32, tag="W1b")
            nc.sync.dma_start(out=W1b[:], in_=W1[P:IN_F, :])
            for w in range(NWIN):
                fa = iopool.tile([P, P], f32, tag="fa")
                nc.sync.dma_start(out=fa[:], in_=featT[0:P, w * P:(w + 1) * P])
                fb = iopool.tile([P, P], f32, tag="fb")
                nc.sync.dma_start(out=fb[:], in_=featT[P:IN_F, w * P:(w + 1) * P])
                ps = pspool.tile([P, HID], f32, tag="d1")
                nc.tensor.matmul(ps[:], lhsT=fa[:], rhs=W1a[:],
                                 start=True, stop=False)
                nc.tensor.matmul(ps[:], lhsT=fb[:], rhs=W1b[:],
                                 start=False, stop=True)
                x1s = iopool.tile([P, HID], f16, tag="x1s")
                nc.scalar.activation(x1s[:], ps[:], AF.Copy)
                nc.sync.dma_start(out=x1[w * P:(w + 1) * P, :], in_=x1s[:])
    nc.compile()
    return nc


def _spmm_phase(nc, tc, mybir, quota, table, layer2, W2t, out, iopool, gpool,
                spool, wpool, pswin, psdense, iota16, idx_all, rl_all,
                val_all):
    """Emit the spmm super-block loop. layer1: hT = relu(msgs.T @ S) then
    x2 = hT.T @ W2 -> out rows (fp16). layer2: o = softmax(relu(S.T @ msgs))
    -> out rows (fp32)."""
    f32, f16, i16 = mybir.dt.float32, mybir.dt.float16, mybir.dt.int16
    AF = mybir.ActivationFunctionType
    ALU = mybir.AluOpType
    import concourse.bass as bass  # noqa: F401

    qs = [int(q) for q in quota]
    T = sum(qs)
    ncall16 = [SB * q * P // 16 for q in qs]

    idxcol = 0
    for sb in range(NSB):
        dsts = []
        for k in range(NCHUNK):
            nci = ncall16[k]
            nidx = SB * qs[k] * P
            it = iopool.tile([P, nci], i16, tag=f"idx{k}")
            nc.sync.dma_start(out=it[:], in_=idx_all[:, idxcol:idxcol + nci])
            dst = gpool.tile([P, SB * qs[k], P], f16, tag=f"gd{k}")
            nc.gpsimd.dma_gather(
                dst[:], table[k * CHROWS:(k + 1) * CHROWS, :],
                it[:], nidx, nidx, P, single_packet=False)
            dsts.append(dst)
            idxcol += nci
        rlt = iopool.tile([P, SB * T], f16, tag="rlt")
        nc.sync.dma_start(out=rlt[:],
                          in_=rl_all[:, sb * SB * T:(sb + 1) * SB * T])
        vlt = iopool.tile([P, SB * T], f16, tag="vlt")
        nc.sync.dma_start(out=vlt[:],
                          in_=val_all[:, sb * SB * T:(sb + 1) * SB * T])

        for wl in range(SB):
            w = sb * SB + wl
            S01 = spool.tile([P, T, P], f16, tag="S01")
            nc.vector.tensor_tensor(
                out=S01[:],
                in0=rlt[:, wl * T:(wl + 1) * T, None].to_broadcast([P, T, P]),
                in1=iota16[:], op=ALU.is_equal)
            S = spool.tile([P, T, P], f16, tag="S")
            nc.vector.tensor_tensor(
                out=S[:], in0=S01[:],
                in1=vlt[:, wl * T:(wl + 1) * T, None].to_broadcast([P, T, P]),
                op=ALU.mult)

            acc = pswin.tile([P, P if not layer2 else OUT], f32, tag="acc")
            j = 0
            for k in range(NCHUNK):
                for t in range(qs[k]):
                    if layer2:
                        nc.tensor.matmul(acc[:], lhsT=S[:, j, :],
                                         rhs=dsts[k][:, wl * qs[k] + t, 0:OUT],
                                         start=(j == 0), stop=(j == T - 1))
                    else:
                        nc.tensor.matmul(acc[:], lhsT=dsts[k][:, wl * qs[k] + t, :],
                                         rhs=S[:, j, :],
                                         start=(j == 0), stop=(j == T - 1))
                    j += 1
            if not layer2:
                hT = wpool.tile([P, P], f32, tag="hT")
                nc.scalar.activation(hT[:], acc[:], AF.Relu)
                x2ps = psdense.tile([P, OUT], f32, tag="d2")
                nc.tensor.matmul(x2ps[:], lhsT=hT[:], rhs=W2t[:],
                                 start=True, stop=True)
                x2s = wpool.tile([P, OUT], f16, tag="x2s")
                nc.scalar.activation(x2s[:], x2ps[:], AF.Copy)
                nc.sync.dma_start(out=out[w * P:(w + 1) * P, :], in_=x2s[:])
            else:
                r = wpool.tile([P, OUT], f32, tag="r")
                nc.scalar.activation(r[:], acc[:], AF.Relu)
                nm = wpool.tile([P, 1], f32, tag="nm")
                nc.vector.tensor_reduce(nm[:], r[:],
                                        axis=mybir.AxisListType.X,
                                        op=ALU.max, negate=True)
                ex = wpool.tile([P, OUT], f32, tag="ex")
                se = wpool.tile([P, 1], f32, tag="se")
                nc.scalar.activation(ex[:], r[:], AF.Exp, bias=nm[:],
                                     accum_out=se[:])
                rs = wpool.tile([P, 1], f32, tag="rs")
                nc.vector.reciprocal(rs[:], se[:])
                o = wpool.tile([P, OUT], f32, tag="o")
                nc.scalar.activation(o[:], ex[:], AF.Copy, scale=rs[:])
                nc.sync.dma_start(out=out[w * P:(w + 1) * P, :], in_=o[:])


def _build_prog_bc(quota, layer2):
    bacc, tile, mybir = _bass_mods()
    f32, f16, i16 = mybir.dt.float32, mybir.dt.float16, mybir.dt.int16

    qs = [int(q) for q in quota]
    T = sum(qs)
    NIDX = NWIN * T * P // 16

    nc = bacc.Bacc("TRN2", target_bir_lowering=False, debug=False,
                   num_devices=NCORES)
    W2 = None
    if layer2:
        table = nc.dram_tensor("x2_full", [NTOT, P], f16,
                               kind="ExternalInput")
        outt = nc.dram_tensor("out", [NPADC, OUT], f32, kind="ExternalOutput")
    else:
        table = nc.dram_tensor("x1_full", [NTOT, HID], f16,
                               kind="ExternalInput")
        outt = nc.dram_tensor("x2", [NPADC, OUT], f16, kind="ExternalOutput")
        W2 = nc.dram_tensor("W2", [HID, OUT], f32, kind="ExternalInput")
    idx_all = nc.dram_tensor("idx_all", [P, NIDX], i16, kind="ExternalInput")
    rl_all = nc.dram_tensor("rl_all", [P, NWIN * T], f16, kind="ExternalInput")
    val_all = nc.dram_tensor("val_all", [P, NWIN * T], f16,
                             kind="ExternalInput")

    with tile.TileContext(nc, num_cores=NCORES) as tc:
        with tc.tile_pool(name="const", bufs=1) as cpool, \
             tc.tile_pool(name="io", bufs=3) as iopool, \
             tc.tile_pool(name="gd", bufs=2) as gpool, \
             tc.tile_pool(name="sblk", bufs=3) as spool, \
             tc.tile_pool(name="wout", bufs=4) as wpool, \
             tc.tile_pool(name="psw", bufs=4, space="PSUM") as pswin, \
             tc.tile_pool(name="psd", bufs=2, space="PSUM") as psdense:
            iota16 = cpool.tile([P, T, P], f16, tag="iota16")
            nc.gpsimd.iota(iota16[:], pattern=[[0, T], [1, P]], base=0,
                           channel_multiplier=0,
                           allow_small_or_imprecise_dtypes=True)
            W2t = None
            if not layer2:
                W2t = cpool.tile([P, OUT], f32, tag="W2t")
                nc.sync.dma_start(out=W2t[:], in_=W2[:])
            _spmm_phase(nc, tc, mybir, qs, table, layer2, W2t, outt,
                        iopool, gpool, spool, wpool, pswin, psdense, iota16,
                        idx_all, rl_all, val_all)
    nc.compile()
    return nc


# ------------------------------------------------------------------- kernel

PROFILE = False          # set True (with NTFF hook installed) to trace launches
LAST_PROFILE = []        # [(exec_time_ns, tmpdir), ...] per launch when PROFILE


def _run(prog, maps, cores):
    from concourse.bass_utils import run_bass_kernel_spmd
    kw = {}
    if PROFILE:
        import tempfile
        kw = dict(trace=True, tmpdir=tempfile.mkdtemp(prefix="gnnprof_"))
    r = run_bass_kernel_spmd(prog, maps, cores, **kw)
    if PROFILE:
        LAST_PROFILE.append((r.exec_time_ns, kw.get("tmpdir")))
    return r


def _get_progs(key):
    if key not in _CACHE:
        _CACHE[key] = (_build_prog_a(), _build_prog_bc(key, False),
                       _build_prog_bc(key, True))
    return _CACHE[key]


def kernel(features, edge_row, edge_col, edge_val, W1, W2):
    features = np.asarray(features, dtype=np.float32)
    quota, edge_maps = _build_edge_inputs(
        np.asarray(edge_row, dtype=np.int64),
        np.asarray(edge_col, dtype=np.int64),
        np.asarray(edge_val, dtype=np.float32))
    key = tuple(int(q) for q in quota)
    prog_a, prog_b, prog_c = _get_progs(key)
    cores = list(range(NCORES))
    W1f = np.ascontiguousarray(W1, dtype=np.float32)
    W2f = np.ascontiguousarray(W2, dtype=np.float32)

    # launch A: dense1
    a_maps = []
    for c in range(NCORES):
        f = np.zeros((NPADC, IN_F), np.float32)
        f[:NSHARD] = features[c * NSHARD:(c + 1) * NSHARD]
        a_maps.append({"featT": np.ascontiguousarray(f.T), "W1": W1f})
    res_a = _run(prog_a, a_maps, cores)
    x1_full = np.concatenate([res_a.results[c]["x1"] for c in range(NCORES)],
                             axis=0)

    # launch B: spmm1 + dense2
    b_maps = [{"x1_full": x1_full, "W2": W2f, **edge_maps[c]}
              for c in range(NCORES)]
    res_b = _run(prog_b, b_maps, cores)
    x2_full = np.zeros((NTOT, P), np.float16)
    x2_full[:, :OUT] = np.concatenate(
        [res_b.results[c]["x2"] for c in range(NCORES)], axis=0)

    # launch C: spmm2 + softmax
    c_maps = [{"x2_full": x2_full, **edge_maps[c]} for c in range(NCORES)]
    res_c = _run(prog_c, c_maps, cores)
    return np.concatenate(
        [res_c.results[c]["out"][:NSHARD] for c in range(NCORES)],
        axis=0).astype(np.float32)



# revision 8
# speedup vs baseline: 2.0667x; 2.0667x over previous
"""Trainium2 Bass kernel for nn_Net_88381837017215 (2-layer GCN message passing).

  h = relu(A @ (features @ W1)); o = softmax(relu(A @ (h @ W2)))

Strategy (8 NeuronCores, SPMD, 3 launches with host gather between):
- Nodes row-sharded: core c owns rows [c*12500,(c+1)*12500), padded to 12544
  (98 windows x 128). Global padded tables: 100352 rows.
- Launch A: x1 = features @ W1 per shard (fp32 PSUM, fp16 out).
  Host concatenates the 8 shards into the full x1 table.
- Launch B: spmm1 + relu + dense2. Edges grouped by owner row-window (128 dst
  nodes) and source-chunk (4 chunks of 25088 table rows so gather indices fit
  int16); each (window,chunk) padded to quota[chunk] tiles of 128 edges. Per
  super-block of SB=7 windows, one bulk dma_gather per chunk fetches the edge
  source rows (fp16, 256B each). Segment-sum via one-hot matmuls:
  S[e,n] = val[e] * (row_local[e]==n) built fp16 with block DVE ops; PE
  accumulates msgs.T @ S into PSUM (output lands transposed, feeding h @ W2
  directly without an explicit transpose). Host concatenates x2 shards.
- Launch C: spmm2 (S.T @ msgs) + relu + on-chip softmax.

kernel(**inputs) takes FULL inputs, shards on host, runs on cores 0-7 via
run_bass_kernel_spmd, returns the FULL [100000, 64] float32 output.
"""
import os
import sys

for _p in ("/opt/trn_rl_repo", "/root/.axon_site/_ro/trn_rl_repo"):
    if os.path.isdir(_p):
        sys.path.insert(0, _p)
        break

import numpy as np

NCORES = 8
N = 100000
P = 128
NSHARD = N // NCORES            # 12500
NWIN = (NSHARD + P - 1) // P    # 98
NPADC = NWIN * P                # 12544
NTOT = NCORES * NPADC           # 100352
NCHUNK = 4
CHROWS = NTOT // NCHUNK         # 25088
SB = 7
NSB = NWIN // SB                # 14
HID, OUT, IN_F = 128, 64, 256


# ---------------------------------------------------------------- host side

def _preprocess(edge_row, edge_col, edge_val):
    core = edge_row // NSHARD
    rlc = edge_row % NSHARD
    win = rlc // P
    row_in_win = rlc % P
    colp = (edge_col // NSHARD) * NPADC + (edge_col % NSHARD)
    chunk = colp // CHROWS
    idx16 = (colp % CHROWS).astype(np.int32)

    key = (core * NWIN + win) * NCHUNK + chunk
    counts = np.bincount(key, minlength=NCORES * NWIN * NCHUNK)
    counts = counts.reshape(NCORES, NWIN, NCHUNK)
    quota = np.ceil(counts.max(axis=(0, 1)) / P).astype(np.int64)
    T = int(quota.sum())

    order = np.argsort(key, kind="stable")
    s_riw = row_in_win[order]
    s_idx = idx16[order]
    s_val = edge_val[order]

    starts = np.zeros(NCORES * NWIN * NCHUNK + 1, np.int64)
    np.cumsum(counts.reshape(-1), out=starts[1:])
    off = np.concatenate([[0], np.cumsum(quota)])
    per_core = []
    for c in range(NCORES):
        idx_arr = np.zeros((NWIN, T, P), np.int16)
        rl_arr = np.zeros((NWIN, T, P), np.float16)
        val_arr = np.zeros((NWIN, T, P), np.float16)
        for w in range(NWIN):
            g0 = (c * NWIN + w) * NCHUNK
            for k in range(NCHUNK):
                a, b = starts[g0 + k], starts[g0 + k + 1]
                n = b - a
                base = int(off[k]) * P
                idx_arr[w].reshape(-1)[base:base + n] = s_idx[a:b]
                rl_arr[w].reshape(-1)[base:base + n] = s_riw[a:b]
                val_arr[w].reshape(-1)[base:base + n] = s_val[a:b]
        per_core.append((idx_arr, rl_arr, val_arr))
    return quota, per_core


def _build_edge_inputs(edge_row, edge_col, edge_val):
    quota, per_core = _preprocess(edge_row, edge_col, edge_val)
    T = int(quota.sum())
    edge_maps = []
    for c in range(NCORES):
        idx_arr, rl_arr, val_arr = per_core[c]
        calls = []
        for sb in range(NSB):
            o = 0
            for k in range(NCHUNK):
                q = int(quota[k])
                blk = idx_arr[sb * SB:(sb + 1) * SB, o:o + q, :]
                calls.append(blk.reshape(-1).reshape(-1, 16).T)
                o += q
        idx_all = np.tile(np.concatenate(calls, axis=1), (8, 1))
        rl_all = np.ascontiguousarray(
            rl_arr.transpose(2, 0, 1).reshape(P, NWIN * T))
        val_all = np.ascontiguousarray(
            val_arr.transpose(2, 0, 1).reshape(P, NWIN * T))
        edge_maps.append({
            "idx_all": np.ascontiguousarray(idx_all, dtype=np.int16),
            "rl_all": rl_all,
            "val_all": val_all,
        })
    return quota, edge_maps


# ------------------------------------------------------------- bass programs

_CACHE = {}


def _bass_mods():
    import concourse.bacc as bacc
    import concourse.tile as tile
    from concourse import mybir
    return bacc, tile, mybir


def _build_prog_a():
    """x1_shard[NPADC, HID] (fp16) = featT.T @ W1 (fp32 accum, fp16 in/out).

    Whole-shard bulk DMAs (2 in, GRP-window out batches) to avoid per-window
    HWDGE issue overhead; PSUM drains alternate scalar/vector engines."""
    bacc, tile, mybir = _bass_mods()
    f32, f16 = mybir.dt.float32, mybir.dt.float16
    AF = mybir.ActivationFunctionType

    nc = bacc.Bacc("TRN2", target_bir_lowering=False, debug=False,
                   num_devices=NCORES)
    featT = nc.dram_tensor("featT", [IN_F, NPADC], f16, kind="ExternalInput")
    W1 = nc.dram_tensor("W1", [IN_F, HID], f16, kind="ExternalInput")
    x1 = nc.dram_tensor("x1", [NPADC, HID], f16, kind="ExternalOutput")
    x1v = x1.rearrange("(w r) c -> w r c", r=P)

    GRP = 14                      # windows per output DMA; 98 = 7 * 14
    with tile.TileContext(nc, num_cores=NCORES) as tc:
        with tc.tile_pool(name="const", bufs=1) as cpool, \
             tc.tile_pool(name="out", bufs=3) as opool, \
             tc.tile_pool(name="ps", bufs=8, space="PSUM") as pspool:
            W1a = cpool.tile([P, HID], f16, tag="W1a")
            nc.sync.dma_start(out=W1a[:], in_=W1[0:P, :])
            W1b = cpool.tile([P, HID], f16, tag="W1b")
            nc.sync.dma_start(out=W1b[:], in_=W1[P:IN_F, :])
            fA = cpool.tile([P, NPADC], f16, tag="fA")
            nc.sync.dma_start(out=fA[:], in_=featT[0:P, :])
            fB = cpool.tile([P, NPADC], f16, tag="fB")
            nc.sync.dma_start(out=fB[:], in_=featT[P:IN_F, :])
            for g in range(NWIN // GRP):
                ot = opool.tile([P, GRP, HID], f16, tag="ot")
                for j in range(GRP):
                    w = g * GRP + j
                    ps = pspool.tile([P, HID], f32, tag="d1")
                    nc.tensor.matmul(ps[:], lhsT=fA[:, w * P:(w + 1) * P],
                                     rhs=W1a[:], start=True, stop=False)
                    nc.tensor.matmul(ps[:], lhsT=fB[:, w * P:(w + 1) * P],
                                     rhs=W1b[:], start=False, stop=True)
                    if j % 2 == 0:
                        nc.scalar.activation(ot[:, j, :], ps[:], AF.Copy)
                    else:
                        nc.vector.tensor_copy(ot[:, j, :], ps[:])
                nc.sync.dma_start(
                    out=x1v[g * GRP:(g + 1) * GRP, :, :].rearrange(
                        "w r c -> r w c"),
                    in_=ot[:])
    nc.compile()
    return nc


def _spmm_phase(nc, tc, mybir, quota, table, layer2, W2t, out, iopool, gpool,
                spool, wpool, pswin, psdense, iota16, idx_all, rl_all,
                val_all):
    """Emit the spmm super-block loop. layer1: hT = relu(msgs.T @ S) then
    x2 = hT.T @ W2 -> out rows (fp16). layer2: o = softmax(relu(S.T @ msgs))
    -> out rows (fp32)."""
    f32, f16, i16 = mybir.dt.float32, mybir.dt.float16, mybir.dt.int16
    AF = mybir.ActivationFunctionType
    ALU = mybir.AluOpType
    import concourse.bass as bass  # noqa: F401

    qs = [int(q) for q in quota]
    T = sum(qs)
    ncall16 = [SB * q * P // 16 for q in qs]

    idxcol = 0
    for sb in range(NSB):
        dsts = []
        for k in range(NCHUNK):
            nci = ncall16[k]
            nidx = SB * qs[k] * P
            it = iopool.tile([P, nci], i16, tag=f"idx{k}")
            nc.sync.dma_start(out=it[:], in_=idx_all[:, idxcol:idxcol + nci])
            dst = gpool.tile([P, SB * qs[k], P], f16, tag=f"gd{k}")
            nc.gpsimd.dma_gather(
                dst[:], table[k * CHROWS:(k + 1) * CHROWS, :],
                it[:], nidx, nidx, P, single_packet=False,
                queue_num=(sb + k) % 4)
            dsts.append(dst)
            idxcol += nci
        rlt = iopool.tile([P, SB * T], f16, tag="rlt")
        nc.sync.dma_start(out=rlt[:],
                          in_=rl_all[:, sb * SB * T:(sb + 1) * SB * T])
        vlt = iopool.tile([P, SB * T], f16, tag="vlt")
        nc.sync.dma_start(out=vlt[:],
                          in_=val_all[:, sb * SB * T:(sb + 1) * SB * T])

        for wl in range(SB):
            w = sb * SB + wl
            S01 = spool.tile([P, T, P], f16, tag="S01")
            nc.vector.tensor_tensor(
                out=S01[:],
                in0=rlt[:, wl * T:(wl + 1) * T, None].to_broadcast([P, T, P]),
                in1=iota16[:], op=ALU.is_equal)
            S = spool.tile([P, T, P], f16, tag="S")
            nc.vector.tensor_tensor(
                out=S[:], in0=S01[:],
                in1=vlt[:, wl * T:(wl + 1) * T, None].to_broadcast([P, T, P]),
                op=ALU.mult)

            acc = pswin.tile([P, P if not layer2 else OUT], f32, tag="acc")
            j = 0
            for k in range(NCHUNK):
                for t in range(qs[k]):
                    if layer2:
                        nc.tensor.matmul(acc[:], lhsT=S[:, j, :],
                                         rhs=dsts[k][:, wl * qs[k] + t, 0:OUT],
                                         start=(j == 0), stop=(j == T - 1))
                    else:
                        nc.tensor.matmul(acc[:], lhsT=dsts[k][:, wl * qs[k] + t, :],
                                         rhs=S[:, j, :],
                                         start=(j == 0), stop=(j == T - 1))
                    j += 1
            if not layer2:
                hT = wpool.tile([P, P], f16, tag="hT")
                nc.scalar.activation(hT[:], acc[:], AF.Relu)
                x2ps = psdense.tile([P, OUT], f32, tag="d2")
                nc.tensor.matmul(x2ps[:], lhsT=hT[:], rhs=W2t[:],
                                 start=True, stop=True)
                x2s = wpool.tile([P, OUT], f16, tag="x2s")
                nc.scalar.activation(x2s[:], x2ps[:], AF.Copy)
                nc.sync.dma_start(out=out[w * P:(w + 1) * P, :], in_=x2s[:])
            else:
                r = wpool.tile([P, OUT], f32, tag="r")
                nc.scalar.activation(r[:], acc[:], AF.Relu)
                nm = wpool.tile([P, 1], f32, tag="nm")
                nc.vector.tensor_reduce(nm[:], r[:],
                                        axis=mybir.AxisListType.X,
                                        op=ALU.max, negate=True)
                ex = wpool.tile([P, OUT], f32, tag="ex")
                se = wpool.tile([P, 1], f32, tag="se")
                nc.scalar.activation(ex[:], r[:], AF.Exp, bias=nm[:],
                                     accum_out=se[:])
                rs = wpool.tile([P, 1], f32, tag="rs")
                nc.vector.reciprocal(rs[:], se[:])
                o = wpool.tile([P, OUT], f32, tag="o")
                nc.scalar.activation(o[:], ex[:], AF.Copy, scale=rs[:])
                nc.sync.dma_start(out=out[w * P:(w + 1) * P, :], in_=o[:])


def _build_prog_bc(quota, layer2):
    bacc, tile, mybir = _bass_mods()
    f32, f16, i16 = mybir.dt.float32, mybir.dt.float16, mybir.dt.int16

    qs = [int(q) for q in quota]
    T = sum(qs)
    NIDX = NWIN * T * P // 16

    nc = bacc.Bacc("TRN2", target_bir_lowering=False, debug=False,
                   num_devices=NCORES, num_swdge_queues=4)
    W2 = None
    if layer2:
        table = nc.dram_tensor("x2_full", [NTOT, P], f16,
                               kind="ExternalInput")
        outt = nc.dram_tensor("out", [NPADC, OUT], f32, kind="ExternalOutput")
    else:
        table = nc.dram_tensor("x1_full", [NTOT, HID], f16,
                               kind="ExternalInput")
        outt = nc.dram_tensor("x2", [NPADC, OUT], f16, kind="ExternalOutput")
        W2 = nc.dram_tensor("W2", [HID, OUT], f16, kind="ExternalInput")
    idx_all = nc.dram_tensor("idx_all", [P, NIDX], i16, kind="ExternalInput")
    rl_all = nc.dram_tensor("rl_all", [P, NWIN * T], f16, kind="ExternalInput")
    val_all = nc.dram_tensor("val_all", [P, NWIN * T], f16,
                             kind="ExternalInput")

    with tile.TileContext(nc, num_cores=NCORES) as tc:
        with tc.tile_pool(name="const", bufs=1) as cpool, \
             tc.tile_pool(name="io", bufs=3) as iopool, \
             tc.tile_pool(name="gd", bufs=2) as gpool, \
             tc.tile_pool(name="sblk", bufs=3) as spool, \
             tc.tile_pool(name="wout", bufs=4) as wpool, \
             tc.tile_pool(name="psw", bufs=4, space="PSUM") as pswin, \
             tc.tile_pool(name="psd", bufs=2, space="PSUM") as psdense:
            iota16 = cpool.tile([P, T, P], f16, tag="iota16")
            nc.gpsimd.iota(iota16[:], pattern=[[0, T], [1, P]], base=0,
                           channel_multiplier=0,
                           allow_small_or_imprecise_dtypes=True)
            W2t = None
            if not layer2:
                W2t = cpool.tile([P, OUT], f16, tag="W2t")
                nc.sync.dma_start(out=W2t[:], in_=W2[:])
            _spmm_phase(nc, tc, mybir, qs, table, layer2, W2t, outt,
                        iopool, gpool, spool, wpool, pswin, psdense, iota16,
                        idx_all, rl_all, val_all)
    nc.compile()
    return nc


# ------------------------------------------------------------------- kernel

PROFILE = False          # set True (with NTFF hook installed) to trace launches
LAST_PROFILE = []        # [(exec_time_ns, tmpdir), ...] per launch when PROFILE


def _run(prog, maps, cores):
    from concourse.bass_utils import run_bass_kernel_spmd
    kw = {}
    if PROFILE:
        import tempfile
        kw = dict(trace=True, tmpdir=tempfile.mkdtemp(prefix="gnnprof_"))
    r = run_bass_kernel_spmd(prog, maps, cores, **kw)
    if PROFILE:
        LAST_PROFILE.append((r.exec_time_ns, kw.get("tmpdir")))
    return r


def _get_progs(key):
    if key not in _CACHE:
        _CACHE[key] = (_build_prog_a(), _build_prog_bc(key, False),
                       _build_prog_bc(key, True))
    return _CACHE[key]


def kernel(features, edge_row, edge_col, edge_val, W1, W2):
    features = np.asarray(features, dtype=np.float32)
    quota, edge_maps = _build_edge_inputs(
        np.asarray(edge_row, dtype=np.int64),
        np.asarray(edge_col, dtype=np.int64),
        np.asarray(edge_val, dtype=np.float32))
    key = tuple(int(q) for q in quota)
    prog_a, prog_b, prog_c = _get_progs(key)
    cores = list(range(NCORES))
    W1f = np.ascontiguousarray(W1, dtype=np.float16)
    W2f = np.ascontiguousarray(W2, dtype=np.float16)

    # launch A: dense1
    a_maps = []
    for c in range(NCORES):
        f = np.zeros((NPADC, IN_F), np.float16)
        f[:NSHARD] = features[c * NSHARD:(c + 1) * NSHARD].astype(np.float16)
        a_maps.append({"featT": np.ascontiguousarray(f.T), "W1": W1f})
    res_a = _run(prog_a, a_maps, cores)
    x1_full = np.concatenate([res_a.results[c]["x1"] for c in range(NCORES)],
                             axis=0)

    # launch B: spmm1 + dense2
    b_maps = [{"x1_full": x1_full, "W2": W2f, **edge_maps[c]}
              for c in range(NCORES)]
    res_b = _run(prog_b, b_maps, cores)
    x2_full = np.zeros((NTOT, P), np.float16)
    x2_full[:, :OUT] = np.concatenate(
        [res_b.results[c]["x2"] for c in range(NCORES)], axis=0)

    # launch C: spmm2 + softmax
    c_maps = [{"x2_full": x2_full, **edge_maps[c]} for c in range(NCORES)]
    res_c = _run(prog_c, c_maps, cores)
    return np.concatenate(
        [res_c.results[c]["out"][:NSHARD] for c in range(NCORES)],
        axis=0).astype(np.float32)



# revision 11
# speedup vs baseline: 2.4900x; 1.2048x over previous
"""Trainium2 Bass kernel for nn_Net_88381837017215 (2-layer GCN message passing).

  h = relu(A @ (features @ W1)); o = softmax(relu(A @ (h @ W2)))

Strategy (8 NeuronCores, SPMD, 3 launches with host gather between):
- Nodes row-sharded: core c owns rows [c*12500,(c+1)*12500), padded to 12544
  (98 windows x 128). Global padded tables: 100352 rows.
- Launch A: x1 = features @ W1 per shard (fp32 PSUM, fp16 out).
  Host concatenates the 8 shards into the full x1 table.
- Launch B: spmm1 + relu + dense2. Edges grouped by owner row-window (128 dst
  nodes) and source-chunk (4 chunks of 25088 table rows so gather indices fit
  int16); each (window,chunk) padded to quota[chunk] tiles of 128 edges. Per
  super-block of SB=7 windows, one bulk dma_gather per chunk fetches the edge
  source rows (fp16, 256B each). Segment-sum via one-hot matmuls:
  S[e,n] = val[e] * (row_local[e]==n) built fp16 with block DVE ops; PE
  accumulates msgs.T @ S into PSUM (output lands transposed, feeding h @ W2
  directly without an explicit transpose). Host concatenates x2 shards.
- Launch C: spmm2 (S.T @ msgs) + relu + on-chip softmax.

kernel(**inputs) takes FULL inputs, shards on host, runs on cores 0-7 via
run_bass_kernel_spmd, returns the FULL [100000, 64] float32 output.
"""
import os
import sys

for _p in ("/opt/trn_rl_repo", "/root/.axon_site/_ro/trn_rl_repo"):
    if os.path.isdir(_p):
        sys.path.insert(0, _p)
        break

import numpy as np

NCORES = 8
N = 100000
P = 128
NSHARD = N // NCORES            # 12500
NWIN = (NSHARD + P - 1) // P    # 98
NPADC = NWIN * P                # 12544
NTOT = NCORES * NPADC           # 100352
NCHUNK = 4
CHROWS = NTOT // NCHUNK         # 25088
SB = 7
NSB = NWIN // SB                # 14
HID, OUT, IN_F = 128, 64, 256


# ---------------------------------------------------------------- host side

def _preprocess(edge_row, edge_col, edge_val):
    core = edge_row // NSHARD
    rlc = edge_row % NSHARD
    win = rlc // P
    row_in_win = rlc % P
    colp = (edge_col // NSHARD) * NPADC + (edge_col % NSHARD)
    chunk = colp // CHROWS
    idx16 = (colp % CHROWS).astype(np.int32)

    key = (core * NWIN + win) * NCHUNK + chunk
    counts = np.bincount(key, minlength=NCORES * NWIN * NCHUNK)
    counts = counts.reshape(NCORES, NWIN, NCHUNK)
    quota = np.ceil(counts.max(axis=(0, 1)) / P).astype(np.int64)
    T = int(quota.sum())

    order = np.argsort(key, kind="stable")
    s_riw = row_in_win[order]
    s_idx = idx16[order]
    s_val = edge_val[order]

    starts = np.zeros(NCORES * NWIN * NCHUNK + 1, np.int64)
    np.cumsum(counts.reshape(-1), out=starts[1:])
    off = np.concatenate([[0], np.cumsum(quota)])
    per_core = []
    for c in range(NCORES):
        idx_arr = np.zeros((NWIN, T, P), np.int16)
        rl_arr = np.zeros((NWIN, T, P), np.int64)
        val_arr = np.zeros((NWIN, T, P), np.float16)
        for w in range(NWIN):
            g0 = (c * NWIN + w) * NCHUNK
            for k in range(NCHUNK):
                a, b = starts[g0 + k], starts[g0 + k + 1]
                n = b - a
                base = int(off[k]) * P
                idx_arr[w].reshape(-1)[base:base + n] = s_idx[a:b]
                rl_arr[w].reshape(-1)[base:base + n] = s_riw[a:b]
                val_arr[w].reshape(-1)[base:base + n] = s_val[a:b]
        per_core.append((idx_arr, rl_arr, val_arr))
    return quota, per_core


def _build_edge_inputs(edge_row, edge_col, edge_val):
    quota, per_core = _preprocess(edge_row, edge_col, edge_val)
    T = int(quota.sum())
    edge_maps = []
    for c in range(NCORES):
        idx_arr, rl_arr, val_arr = per_core[c]
        calls = []
        for sb in range(NSB):
            o = 0
            for k in range(NCHUNK):
                q = int(quota[k])
                blk = idx_arr[sb * SB:(sb + 1) * SB, o:o + q, :]
                calls.append(blk.reshape(-1).reshape(-1, 16).T)
                o += q
        idx_all = np.tile(np.concatenate(calls, axis=1), (8, 1))
        # Dense one-hot S tiles built on host: S[w, t, e, n] = val (n == rl).
        # Padded slots have val 0. Uploaded instead of building on DVE
        # (frees the vector engine and removes GpSimd SBUF-port contention).
        S4 = np.zeros((NWIN, T, P, P), np.float16)
        np.put_along_axis(S4, rl_arr[..., None], val_arr[..., None], axis=3)
        s_all = np.ascontiguousarray(
            S4.transpose(2, 0, 1, 3).reshape(P, NWIN * T * P))
        edge_maps.append({
            "idx_all": np.ascontiguousarray(idx_all, dtype=np.int16),
            "s_all": s_all,
        })
    return quota, edge_maps


# ------------------------------------------------------------- bass programs

_CACHE = {}


def _bass_mods():
    import concourse.bacc as bacc
    import concourse.tile as tile
    from concourse import mybir
    return bacc, tile, mybir


def _build_prog_a():
    """x1_shard[NPADC, HID] (fp16) = featT.T @ W1 (fp32 accum, fp16 in/out).

    Whole-shard bulk DMAs (2 in, GRP-window out batches) to avoid per-window
    HWDGE issue overhead; PSUM drains alternate scalar/vector engines."""
    bacc, tile, mybir = _bass_mods()
    f32, f16 = mybir.dt.float32, mybir.dt.float16
    AF = mybir.ActivationFunctionType

    nc = bacc.Bacc("TRN2", target_bir_lowering=False, debug=False,
                   num_devices=NCORES)
    featT = nc.dram_tensor("featT", [IN_F, NPADC], f16, kind="ExternalInput")
    W1 = nc.dram_tensor("W1", [IN_F, HID], f16, kind="ExternalInput")
    x1 = nc.dram_tensor("x1", [NPADC, HID], f16, kind="ExternalOutput")
    x1v = x1.rearrange("(w r) c -> w r c", r=P)

    GRP = 14                      # windows per output DMA; 98 = 7 * 14
    with tile.TileContext(nc, num_cores=NCORES) as tc:
        with tc.tile_pool(name="const", bufs=1) as cpool, \
             tc.tile_pool(name="out", bufs=3) as opool, \
             tc.tile_pool(name="ps", bufs=8, space="PSUM") as pspool:
            W1a = cpool.tile([P, HID], f16, tag="W1a")
            nc.sync.dma_start(out=W1a[:], in_=W1[0:P, :])
            W1b = cpool.tile([P, HID], f16, tag="W1b")
            nc.sync.dma_start(out=W1b[:], in_=W1[P:IN_F, :])
            fA = cpool.tile([P, NPADC], f16, tag="fA")
            nc.sync.dma_start(out=fA[:], in_=featT[0:P, :])
            fB = cpool.tile([P, NPADC], f16, tag="fB")
            nc.sync.dma_start(out=fB[:], in_=featT[P:IN_F, :])
            for g in range(NWIN // GRP):
                ot = opool.tile([P, GRP, HID], f16, tag="ot")
                for j in range(GRP):
                    w = g * GRP + j
                    ps = pspool.tile([P, HID], f32, tag="d1")
                    nc.tensor.matmul(ps[:], lhsT=fA[:, w * P:(w + 1) * P],
                                     rhs=W1a[:], start=True, stop=False)
                    nc.tensor.matmul(ps[:], lhsT=fB[:, w * P:(w + 1) * P],
                                     rhs=W1b[:], start=False, stop=True)
                    if j % 2 == 0:
                        nc.scalar.activation(ot[:, j, :], ps[:], AF.Copy)
                    else:
                        nc.vector.tensor_copy(ot[:, j, :], ps[:])
                nc.sync.dma_start(
                    out=x1v[g * GRP:(g + 1) * GRP, :, :].rearrange(
                        "w r c -> r w c"),
                    in_=ot[:])
    nc.compile()
    return nc


def _spmm_phase(nc, tc, mybir, quota, table, layer2, W2t, out, iopool, gpool,
                spool, wpool, pswin, psdense, idx_all, s_all):
    """Emit the spmm super-block loop. layer1: hT = relu(msgs.T @ S) then
    x2 = hT.T @ W2 -> out rows (fp16). layer2: o = softmax(relu(S.T @ msgs))
    -> out rows (fp32). S tiles are host-precomputed and streamed in."""
    f32, f16, i16 = mybir.dt.float32, mybir.dt.float16, mybir.dt.int16
    AF = mybir.ActivationFunctionType
    ALU = mybir.AluOpType

    qs = [int(q) for q in quota]
    T = sum(qs)
    ncall16 = [SB * q * P // 16 for q in qs]
    outw = OUT if layer2 else OUT
    outv = out.rearrange("(w r) c -> w r c", r=P)

    idxcol = 0
    for sb in range(NSB):
        dsts = []
        for k in range(NCHUNK):
            nci = ncall16[k]
            nidx = SB * qs[k] * P
            it = iopool.tile([P, nci], i16, tag=f"idx{k}")
            nc.sync.dma_start(out=it[:], in_=idx_all[:, idxcol:idxcol + nci])
            dst = gpool.tile([P, SB * qs[k], P], f16, tag=f"gd{k}")
            nc.gpsimd.dma_gather(
                dst[:], table[k * CHROWS:(k + 1) * CHROWS, :],
                it[:], nidx, nidx, P, single_packet=False,
                queue_num=(sb + k) % 4)
            dsts.append(dst)
            idxcol += nci
        St = spool.tile([P, SB, T, P], f16, tag="St")
        nc.sync.dma_start(
            out=St[:], in_=s_all[:, sb * SB * T * P:(sb + 1) * SB * T * P])

        ot = wpool.tile([P, SB, outw], f32 if layer2 else f16, tag="ot")
        for wl in range(SB):
            acc = pswin.tile([P, P if not layer2 else OUT], f32, tag="acc")
            j = 0
            for k in range(NCHUNK):
                for t in range(qs[k]):
                    if layer2:
                        nc.tensor.matmul(acc[:], lhsT=St[:, wl, j, :],
                                         rhs=dsts[k][:, wl * qs[k] + t, 0:OUT],
                                         start=(j == 0), stop=(j == T - 1))
                    else:
                        nc.tensor.matmul(acc[:], lhsT=dsts[k][:, wl * qs[k] + t, :],
                                         rhs=St[:, wl, j, :],
                                         start=(j == 0), stop=(j == T - 1))
                    j += 1
            if not layer2:
                hT = wpool.tile([P, P], f16, tag="hT")
                nc.scalar.activation(hT[:], acc[:], AF.Relu)
                x2ps = psdense.tile([P, OUT], f32, tag="d2")
                nc.tensor.matmul(x2ps[:], lhsT=hT[:], rhs=W2t[:],
                                 start=True, stop=True)
                nc.scalar.activation(ot[:, wl, :], x2ps[:], AF.Copy)
            else:
                r = wpool.tile([P, OUT], f32, tag="r")
                nc.scalar.activation(r[:], acc[:], AF.Relu)
                nm = wpool.tile([P, 1], f32, tag="nm")
                nc.vector.tensor_reduce(nm[:], r[:],
                                        axis=mybir.AxisListType.X,
                                        op=ALU.max, negate=True)
                ex = wpool.tile([P, OUT], f32, tag="ex")
                se = wpool.tile([P, 1], f32, tag="se")
                nc.scalar.activation(ex[:], r[:], AF.Exp, bias=nm[:],
                                     accum_out=se[:])
                rs = wpool.tile([P, 1], f32, tag="rs")
                nc.vector.reciprocal(rs[:], se[:])
                nc.scalar.activation(ot[:, wl, :], ex[:], AF.Copy, scale=rs[:])
        nc.sync.dma_start(
            out=outv[sb * SB:(sb + 1) * SB, :, :].rearrange("w r c -> r w c"),
            in_=ot[:])


def _build_prog_bc(quota, layer2):
    bacc, tile, mybir = _bass_mods()
    f32, f16, i16 = mybir.dt.float32, mybir.dt.float16, mybir.dt.int16

    qs = [int(q) for q in quota]
    T = sum(qs)
    NIDX = NWIN * T * P // 16

    nc = bacc.Bacc("TRN2", target_bir_lowering=False, debug=False,
                   num_devices=NCORES, num_swdge_queues=4)
    W2 = None
    if layer2:
        table = nc.dram_tensor("x2_full", [NTOT, P], f16,
                               kind="ExternalInput")
        outt = nc.dram_tensor("out", [NPADC, OUT], f32, kind="ExternalOutput")
    else:
        table = nc.dram_tensor("x1_full", [NTOT, HID], f16,
                               kind="ExternalInput")
        outt = nc.dram_tensor("x2", [NPADC, OUT], f16, kind="ExternalOutput")
        W2 = nc.dram_tensor("W2", [HID, OUT], f16, kind="ExternalInput")
    idx_all = nc.dram_tensor("idx_all", [P, NIDX], i16, kind="ExternalInput")
    s_all = nc.dram_tensor("s_all", [P, NWIN * T * P], f16,
                           kind="ExternalInput")

    with tile.TileContext(nc, num_cores=NCORES) as tc:
        with tc.tile_pool(name="const", bufs=1) as cpool, \
             tc.tile_pool(name="io", bufs=3) as iopool, \
             tc.tile_pool(name="gd", bufs=2) as gpool, \
             tc.tile_pool(name="sblk", bufs=2) as spool, \
             tc.tile_pool(name="wout", bufs=3) as wpool, \
             tc.tile_pool(name="psw", bufs=4, space="PSUM") as pswin, \
             tc.tile_pool(name="psd", bufs=2, space="PSUM") as psdense:
            W2t = None
            if not layer2:
                W2t = cpool.tile([P, OUT], f16, tag="W2t")
                nc.sync.dma_start(out=W2t[:], in_=W2[:])
            _spmm_phase(nc, tc, mybir, qs, table, layer2, W2t, outt,
                        iopool, gpool, spool, wpool, pswin, psdense,
                        idx_all, s_all)
    nc.compile()
    return nc


# ------------------------------------------------------------------- kernel

PROFILE = False          # set True (with NTFF hook installed) to trace launches
LAST_PROFILE = []        # [(exec_time_ns, tmpdir), ...] per launch when PROFILE


def _run(prog, maps, cores):
    from concourse.bass_utils import run_bass_kernel_spmd
    kw = {}
    if PROFILE:
        import tempfile
        kw = dict(trace=True, tmpdir=tempfile.mkdtemp(prefix="gnnprof_"))
    r = run_bass_kernel_spmd(prog, maps, cores, **kw)
    if PROFILE:
        LAST_PROFILE.append((r.exec_time_ns, kw.get("tmpdir")))
    return r


def _get_progs(key):
    if key not in _CACHE:
        _CACHE[key] = (_build_prog_a(), _build_prog_bc(key, False),
                       _build_prog_bc(key, True))
    return _CACHE[key]


def kernel(features, edge_row, edge_col, edge_val, W1, W2):
    features = np.asarray(features, dtype=np.float32)
    quota, edge_maps = _build_edge_inputs(
        np.asarray(edge_row, dtype=np.int64),
        np.asarray(edge_col, dtype=np.int64),
        np.asarray(edge_val, dtype=np.float32))
    key = tuple(int(q) for q in quota)
    prog_a, prog_b, prog_c = _get_progs(key)
    cores = list(range(NCORES))
    W1f = np.ascontiguousarray(W1, dtype=np.float16)
    W2f = np.ascontiguousarray(W2, dtype=np.float16)

    # launch A: dense1
    a_maps = []
    for c in range(NCORES):
        f = np.zeros((NPADC, IN_F), np.float16)
        f[:NSHARD] = features[c * NSHARD:(c + 1) * NSHARD].astype(np.float16)
        a_maps.append({"featT": np.ascontiguousarray(f.T), "W1": W1f})
    res_a = _run(prog_a, a_maps, cores)
    x1_full = np.concatenate([res_a.results[c]["x1"] for c in range(NCORES)],
                             axis=0)

    # launch B: spmm1 + dense2
    b_maps = [{"x1_full": x1_full, "W2": W2f, **edge_maps[c]}
              for c in range(NCORES)]
    res_b = _run(prog_b, b_maps, cores)
    x2_full = np.zeros((NTOT, P), np.float16)
    x2_full[:, :OUT] = np.concatenate(
        [res_b.results[c]["x2"] for c in range(NCORES)], axis=0)

    # launch C: spmm2 + softmax
    c_maps = [{"x2_full": x2_full, **edge_maps[c]} for c in range(NCORES)]
    res_c = _run(prog_c, c_maps, cores)
    return np.concatenate(
        [res_c.results[c]["out"][:NSHARD] for c in range(NCORES)],
        axis=0).astype(np.float32)



# revision 13
# speedup vs baseline: 2.7339x; 1.0979x over previous
"""Trainium2 Bass kernel for nn_Net_88381837017215 (2-layer GCN message passing).

  h = relu(A @ (features @ W1)); o = softmax(relu(A @ (h @ W2)))

Strategy (8 NeuronCores, SPMD, 3 launches with host gather between):
- Nodes row-sharded: core c owns rows [c*12500,(c+1)*12500), padded to 12544
  (98 windows x 128). Global padded tables: 100352 rows.
- Launch A: x1 = features @ W1 per shard (fp32 PSUM, fp16 out).
  Host concatenates the 8 shards into the full x1 table.
- Launch B: spmm1 + relu + dense2. Edges grouped by owner row-window (128 dst
  nodes) and source-chunk (4 chunks of 25088 table rows so gather indices fit
  int16); each (window,chunk) padded to quota[chunk] tiles of 128 edges. Per
  super-block of SB=7 windows, one bulk dma_gather per chunk fetches the edge
  source rows (fp16, 256B each). Segment-sum via one-hot matmuls:
  S[e,n] = val[e] * (row_local[e]==n) built fp16 with block DVE ops; PE
  accumulates msgs.T @ S into PSUM (output lands transposed, feeding h @ W2
  directly without an explicit transpose). Host concatenates x2 shards.
- Launch C: spmm2 (S.T @ msgs) + relu + on-chip softmax.

kernel(**inputs) takes FULL inputs, shards on host, runs on cores 0-7 via
run_bass_kernel_spmd, returns the FULL [100000, 64] float32 output.
"""
import os
import sys

for _p in ("/opt/trn_rl_repo", "/root/.axon_site/_ro/trn_rl_repo"):
    if os.path.isdir(_p):
        sys.path.insert(0, _p)
        break

import numpy as np

NCORES = 8
N = 100000
P = 128
NSHARD = N // NCORES            # 12500
NWIN = (NSHARD + P - 1) // P    # 98
NPADC = NWIN * P                # 12544
NTOT = NCORES * NPADC           # 100352
NCHUNK = 4
CHROWS = NTOT // NCHUNK         # 25088
SB = 7
NSB = NWIN // SB                # 14
HID, OUT, IN_F = 128, 64, 256


# ---------------------------------------------------------------- host side

def _preprocess(edge_row, edge_col, edge_val):
    core = edge_row // NSHARD
    rlc = edge_row % NSHARD
    win = rlc // P
    row_in_win = rlc % P
    colp = (edge_col // NSHARD) * NPADC + (edge_col % NSHARD)
    chunk = colp // CHROWS
    idx16 = (colp % CHROWS).astype(np.int32)

    key = (core * NWIN + win) * NCHUNK + chunk
    counts = np.bincount(key, minlength=NCORES * NWIN * NCHUNK)
    counts = counts.reshape(NCORES, NWIN, NCHUNK)
    quota = np.ceil(counts.max(axis=(0, 1)) / P).astype(np.int64)
    T = int(quota.sum())

    order = np.argsort(key, kind="stable")
    s_riw = row_in_win[order]
    s_idx = idx16[order]
    s_val = edge_val[order]

    starts = np.zeros(NCORES * NWIN * NCHUNK + 1, np.int64)
    np.cumsum(counts.reshape(-1), out=starts[1:])
    off = np.concatenate([[0], np.cumsum(quota)])
    per_core = []
    for c in range(NCORES):
        idx_arr = np.zeros((NWIN, T, P), np.int16)
        rl_arr = np.zeros((NWIN, T, P), np.int64)
        val_arr = np.zeros((NWIN, T, P), np.float16)
        for w in range(NWIN):
            g0 = (c * NWIN + w) * NCHUNK
            for k in range(NCHUNK):
                a, b = starts[g0 + k], starts[g0 + k + 1]
                n = b - a
                base = int(off[k]) * P
                idx_arr[w].reshape(-1)[base:base + n] = s_idx[a:b]
                rl_arr[w].reshape(-1)[base:base + n] = s_riw[a:b]
                val_arr[w].reshape(-1)[base:base + n] = s_val[a:b]
        per_core.append((idx_arr, rl_arr, val_arr))
    return quota, per_core


def _build_edge_inputs(edge_row, edge_col, edge_val):
    quota, per_core = _preprocess(edge_row, edge_col, edge_val)
    T = int(quota.sum())
    edge_maps = []
    for c in range(NCORES):
        idx_arr, rl_arr, val_arr = per_core[c]
        calls = []
        for sb in range(NSB):
            o = 0
            for k in range(NCHUNK):
                q = int(quota[k])
                blk = idx_arr[sb * SB:(sb + 1) * SB, o:o + q, :]
                calls.append(blk.reshape(-1).reshape(-1, 16).T)
                o += q
        idx_all = np.tile(np.concatenate(calls, axis=1), (8, 1))
        # Dense one-hot S tiles built on host: S[w, t, e, n] = val (n == rl).
        # Padded slots have val 0. Uploaded instead of building on DVE
        # (frees the vector engine and removes GpSimd SBUF-port contention).
        S4 = np.zeros((NWIN, T, P, P), np.float16)
        np.put_along_axis(S4, rl_arr[..., None], val_arr[..., None], axis=3)
        s_all = np.ascontiguousarray(
            S4.transpose(2, 0, 1, 3).reshape(P, NWIN * T * P))
        edge_maps.append({
            "idx_all": np.ascontiguousarray(idx_all, dtype=np.int16),
            "s_all": s_all,
        })
    return quota, edge_maps


# ------------------------------------------------------------- bass programs

_CACHE = {}


def _bass_mods():
    import concourse.bacc as bacc
    import concourse.tile as tile
    from concourse import mybir
    return bacc, tile, mybir


def _build_prog_a():
    """x1_shard[NPADC, HID] (fp16) = featT.T @ W1 (fp32 accum, fp16 in/out).

    Whole-shard bulk DMAs (2 in, GRP-window out batches) to avoid per-window
    HWDGE issue overhead; PSUM drains alternate scalar/vector engines."""
    bacc, tile, mybir = _bass_mods()
    f32, f16 = mybir.dt.float32, mybir.dt.float16
    AF = mybir.ActivationFunctionType

    nc = bacc.Bacc("TRN2", target_bir_lowering=False, debug=False,
                   num_devices=NCORES)
    featT = nc.dram_tensor("featT", [IN_F, NPADC], f16, kind="ExternalInput")
    W1 = nc.dram_tensor("W1", [IN_F, HID], f16, kind="ExternalInput")
    x1 = nc.dram_tensor("x1", [NPADC, HID], f16, kind="ExternalOutput")
    x1v = x1.rearrange("(w r) c -> w r c", r=P)

    GRP = 14                      # windows per output DMA; 98 = 7 * 14
    with tile.TileContext(nc, num_cores=NCORES) as tc:
        with tc.tile_pool(name="const", bufs=1) as cpool, \
             tc.tile_pool(name="out", bufs=3) as opool, \
             tc.tile_pool(name="ps", bufs=8, space="PSUM") as pspool:
            W1a = cpool.tile([P, HID], f16, tag="W1a")
            nc.sync.dma_start(out=W1a[:], in_=W1[0:P, :])
            W1b = cpool.tile([P, HID], f16, tag="W1b")
            nc.sync.dma_start(out=W1b[:], in_=W1[P:IN_F, :])
            fA = cpool.tile([P, NPADC], f16, tag="fA")
            nc.sync.dma_start(out=fA[:], in_=featT[0:P, :])
            fB = cpool.tile([P, NPADC], f16, tag="fB")
            nc.sync.dma_start(out=fB[:], in_=featT[P:IN_F, :])
            for g in range(NWIN // GRP):
                ot = opool.tile([P, GRP, HID], f16, tag="ot")
                for j in range(GRP):
                    w = g * GRP + j
                    ps = pspool.tile([P, HID], f32, tag="d1")
                    nc.tensor.matmul(ps[:], lhsT=fA[:, w * P:(w + 1) * P],
                                     rhs=W1a[:], start=True, stop=False)
                    nc.tensor.matmul(ps[:], lhsT=fB[:, w * P:(w + 1) * P],
                                     rhs=W1b[:], start=False, stop=True)
                    if j % 2 == 0:
                        nc.scalar.activation(ot[:, j, :], ps[:], AF.Copy)
                    else:
                        nc.vector.tensor_copy(ot[:, j, :], ps[:])
                nc.sync.dma_start(
                    out=x1v[g * GRP:(g + 1) * GRP, :, :].rearrange(
                        "w r c -> r w c"),
                    in_=ot[:])
    nc.compile()
    return nc


def _spmm_phase(nc, tc, mybir, quota, table, layer2, W2t, out, iopool, gpool,
                spool, wpool, pswin, psdense, idx_all, s_all):
    """Emit the spmm super-block loop. layer1: hT = relu(msgs.T @ S) then
    x2 = hT.T @ W2 -> out rows (fp16). layer2: o = softmax(relu(S.T @ msgs))
    -> out rows (fp32). S tiles are host-precomputed and streamed in."""
    f32, f16, i16 = mybir.dt.float32, mybir.dt.float16, mybir.dt.int16
    AF = mybir.ActivationFunctionType
    ALU = mybir.AluOpType

    qs = [int(q) for q in quota]
    T = sum(qs)
    ncall16 = [SB * q * P // 16 for q in qs]
    outw = OUT if layer2 else OUT
    outv = out.rearrange("(w r) c -> w r c", r=P)

    idxcol = 0
    for sb in range(NSB):
        dsts = []
        for k in range(NCHUNK):
            nci = ncall16[k]
            nidx = SB * qs[k] * P
            it = iopool.tile([P, nci], i16, tag=f"idx{k}")
            nc.sync.dma_start(out=it[:], in_=idx_all[:, idxcol:idxcol + nci])
            dst = gpool.tile([P, SB * qs[k], P], f16, tag=f"gd{k}")
            nc.gpsimd.dma_gather(
                dst[:], table[k * CHROWS:(k + 1) * CHROWS, :],
                it[:], nidx, nidx, P, single_packet=False,
                queue_num=(sb + k) % 4)
            dsts.append(dst)
            idxcol += nci
        ot = wpool.tile([P, SB, outw], f32 if layer2 else f16, tag="ot")
        for wl in range(SB):
            w = sb * SB + wl
            St = spool.tile([P, T, P], f16, tag="St")
            nc.sync.dma_start(
                out=St[:], in_=s_all[:, w * T * P:(w + 1) * T * P])
            acc = pswin.tile([P, P if not layer2 else OUT], f32, tag="acc")
            j = 0
            for k in range(NCHUNK):
                for t in range(qs[k]):
                    if layer2:
                        nc.tensor.matmul(acc[:], lhsT=St[:, j, :],
                                         rhs=dsts[k][:, wl * qs[k] + t, 0:OUT],
                                         start=(j == 0), stop=(j == T - 1))
                    else:
                        nc.tensor.matmul(acc[:], lhsT=dsts[k][:, wl * qs[k] + t, :],
                                         rhs=St[:, j, :],
                                         start=(j == 0), stop=(j == T - 1))
                    j += 1
            if not layer2:
                hT = wpool.tile([P, P], f16, tag="hT")
                nc.scalar.activation(hT[:], acc[:], AF.Relu)
                x2ps = psdense.tile([P, OUT], f32, tag="d2")
                nc.tensor.matmul(x2ps[:], lhsT=hT[:], rhs=W2t[:],
                                 start=True, stop=True)
                nc.scalar.activation(ot[:, wl, :], x2ps[:], AF.Copy)
            else:
                r = wpool.tile([P, OUT], f32, tag="r")
                nc.scalar.activation(r[:], acc[:], AF.Relu)
                nm = wpool.tile([P, 1], f32, tag="nm")
                nc.vector.tensor_reduce(nm[:], r[:],
                                        axis=mybir.AxisListType.X,
                                        op=ALU.max, negate=True)
                ex = wpool.tile([P, OUT], f32, tag="ex")
                se = wpool.tile([P, 1], f32, tag="se")
                nc.scalar.activation(ex[:], r[:], AF.Exp, bias=nm[:],
                                     accum_out=se[:])
                rs = wpool.tile([P, 1], f32, tag="rs")
                nc.vector.reciprocal(rs[:], se[:])
                nc.scalar.activation(ot[:, wl, :], ex[:], AF.Copy, scale=rs[:])
        nc.sync.dma_start(
            out=outv[sb * SB:(sb + 1) * SB, :, :].rearrange("w r c -> r w c"),
            in_=ot[:])


def _build_prog_bc(quota, layer2):
    bacc, tile, mybir = _bass_mods()
    f32, f16, i16 = mybir.dt.float32, mybir.dt.float16, mybir.dt.int16

    qs = [int(q) for q in quota]
    T = sum(qs)
    NIDX = NWIN * T * P // 16

    nc = bacc.Bacc("TRN2", target_bir_lowering=False, debug=False,
                   num_devices=NCORES, num_swdge_queues=4)
    W2 = None
    if layer2:
        table = nc.dram_tensor("x2_full", [NTOT, P], f16,
                               kind="ExternalInput")
        outt = nc.dram_tensor("out", [NPADC, OUT], f32, kind="ExternalOutput")
    else:
        table = nc.dram_tensor("x1_full", [NTOT, HID], f16,
                               kind="ExternalInput")
        outt = nc.dram_tensor("x2", [NPADC, OUT], f16, kind="ExternalOutput")
        W2 = nc.dram_tensor("W2", [HID, OUT], f16, kind="ExternalInput")
    idx_all = nc.dram_tensor("idx_all", [P, NIDX], i16, kind="ExternalInput")
    s_all = nc.dram_tensor("s_all", [P, NWIN * T * P], f16,
                           kind="ExternalInput")

    with tile.TileContext(nc, num_cores=NCORES) as tc:
        with tc.tile_pool(name="const", bufs=1) as cpool, \
             tc.tile_pool(name="io", bufs=4) as iopool, \
             tc.tile_pool(name="gd", bufs=3) as gpool, \
             tc.tile_pool(name="sblk", bufs=8) as spool, \
             tc.tile_pool(name="wout", bufs=3) as wpool, \
             tc.tile_pool(name="psw", bufs=4, space="PSUM") as pswin, \
             tc.tile_pool(name="psd", bufs=2, space="PSUM") as psdense:
            W2t = None
            if not layer2:
                W2t = cpool.tile([P, OUT], f16, tag="W2t")
                nc.sync.dma_start(out=W2t[:], in_=W2[:])
            _spmm_phase(nc, tc, mybir, qs, table, layer2, W2t, outt,
                        iopool, gpool, spool, wpool, pswin, psdense,
                        idx_all, s_all)
    nc.compile()
    return nc


# ------------------------------------------------------------------- kernel

PROFILE = False          # set True (with NTFF hook installed) to trace launches
LAST_PROFILE = []        # [(exec_time_ns, tmpdir), ...] per launch when PROFILE


def _run(prog, maps, cores):
    from concourse.bass_utils import run_bass_kernel_spmd
    kw = {}
    if PROFILE:
        import tempfile
        kw = dict(trace=True, tmpdir=tempfile.mkdtemp(prefix="gnnprof_"))
    r = run_bass_kernel_spmd(prog, maps, cores, **kw)
    if PROFILE:
        LAST_PROFILE.append((r.exec_time_ns, kw.get("tmpdir")))
    return r


def _get_progs(key):
    if key not in _CACHE:
        _CACHE[key] = (_build_prog_a(), _build_prog_bc(key, False),
                       _build_prog_bc(key, True))
    return _CACHE[key]


def kernel(features, edge_row, edge_col, edge_val, W1, W2):
    features = np.asarray(features, dtype=np.float32)
    quota, edge_maps = _build_edge_inputs(
        np.asarray(edge_row, dtype=np.int64),
        np.asarray(edge_col, dtype=np.int64),
        np.asarray(edge_val, dtype=np.float32))
    key = tuple(int(q) for q in quota)
    prog_a, prog_b, prog_c = _get_progs(key)
    cores = list(range(NCORES))
    W1f = np.ascontiguousarray(W1, dtype=np.float16)
    W2f = np.ascontiguousarray(W2, dtype=np.float16)

    # launch A: dense1
    a_maps = []
    for c in range(NCORES):
        f = np.zeros((NPADC, IN_F), np.float16)
        f[:NSHARD] = features[c * NSHARD:(c + 1) * NSHARD].astype(np.float16)
        a_maps.append({"featT": np.ascontiguousarray(f.T), "W1": W1f})
    res_a = _run(prog_a, a_maps, cores)
    x1_full = np.concatenate([res_a.results[c]["x1"] for c in range(NCORES)],
                             axis=0)

    # launch B: spmm1 + dense2
    b_maps = [{"x1_full": x1_full, "W2": W2f, **edge_maps[c]}
              for c in range(NCORES)]
    res_b = _run(prog_b, b_maps, cores)
    x2_full = np.zeros((NTOT, P), np.float16)
    x2_full[:, :OUT] = np.concatenate(
        [res_b.results[c]["x2"] for c in range(NCORES)], axis=0)

    # launch C: spmm2 + softmax
    c_maps = [{"x2_full": x2_full, **edge_maps[c]} for c in range(NCORES)]
    res_c = _run(prog_c, c_maps, cores)
    return np.concatenate(
        [res_c.results[c]["out"][:NSHARD] for c in range(NCORES)],
        axis=0).astype(np.float32)



# revision 14
# speedup vs baseline: 3.3665x; 1.2314x over previous
"""Trainium2 Bass kernel for nn_Net_88381837017215 (2-layer GCN message passing).

  h = relu(A @ (features @ W1)); o = softmax(relu(A @ (h @ W2)))

Strategy (8 NeuronCores, SPMD, 3 launches with host gather between):
- Nodes row-sharded: core c owns rows [c*12500,(c+1)*12500), padded to 12544
  (98 windows x 128). Global padded tables: 100352 rows.
- Launch A: x1 = features @ W1 per shard (fp32 PSUM, fp16 out).
  Host concatenates the 8 shards into the full x1 table.
- Launch B: spmm1 + relu + dense2. Edges grouped by owner row-window (128 dst
  nodes) and source-chunk (4 chunks of 25088 table rows so gather indices fit
  int16); each (window,chunk) padded to quota[chunk] tiles of 128 edges. Per
  super-block of SB=7 windows, one bulk dma_gather per chunk fetches the edge
  source rows (fp16, 256B each). Segment-sum via one-hot matmuls:
  S[e,n] = val[e] * (row_local[e]==n) built fp16 with block DVE ops; PE
  accumulates msgs.T @ S into PSUM (output lands transposed, feeding h @ W2
  directly without an explicit transpose). Host concatenates x2 shards.
- Launch C: spmm2 (S.T @ msgs) + relu + on-chip softmax.

kernel(**inputs) takes FULL inputs, shards on host, runs on cores 0-7 via
run_bass_kernel_spmd, returns the FULL [100000, 64] float32 output.
"""
import os
import sys

for _p in ("/opt/trn_rl_repo", "/root/.axon_site/_ro/trn_rl_repo"):
    if os.path.isdir(_p):
        sys.path.insert(0, _p)
        break

import numpy as np

NCORES = 8
N = 100000
P = 128
NSHARD = N // NCORES            # 12500
NWIN = (NSHARD + P - 1) // P    # 98
NPADC = NWIN * P                # 12544
NTOT = NCORES * NPADC           # 100352
NCHUNK = 4
CHROWS = NTOT // NCHUNK         # 25088
SB = 7
NSB = NWIN // SB                # 14
HID, OUT, IN_F = 128, 64, 256


# ---------------------------------------------------------------- host side

def _preprocess(edge_row, edge_col, edge_val):
    core = edge_row // NSHARD
    rlc = edge_row % NSHARD
    win = rlc // P
    row_in_win = rlc % P
    colp = (edge_col // NSHARD) * NPADC + (edge_col % NSHARD)
    chunk = colp // CHROWS
    idx16 = (colp % CHROWS).astype(np.int32)

    key = (core * NWIN + win) * NCHUNK + chunk
    counts = np.bincount(key, minlength=NCORES * NWIN * NCHUNK)
    counts = counts.reshape(NCORES, NWIN, NCHUNK)
    quota = np.ceil(counts.max(axis=(0, 1)) / P).astype(np.int64)
    T = int(quota.sum())

    order = np.argsort(key, kind="stable")
    s_riw = row_in_win[order]
    s_idx = idx16[order]
    s_val = edge_val[order]

    starts = np.zeros(NCORES * NWIN * NCHUNK + 1, np.int64)
    np.cumsum(counts.reshape(-1), out=starts[1:])
    off = np.concatenate([[0], np.cumsum(quota)])
    per_core = []
    for c in range(NCORES):
        idx_arr = np.zeros((NWIN, T, P), np.int16)
        rl_arr = np.zeros((NWIN, T, P), np.int64)
        val_arr = np.zeros((NWIN, T, P), np.float16)
        for w in range(NWIN):
            g0 = (c * NWIN + w) * NCHUNK
            for k in range(NCHUNK):
                a, b = starts[g0 + k], starts[g0 + k + 1]
                n = b - a
                base = int(off[k]) * P
                idx_arr[w].reshape(-1)[base:base + n] = s_idx[a:b]
                rl_arr[w].reshape(-1)[base:base + n] = s_riw[a:b]
                val_arr[w].reshape(-1)[base:base + n] = s_val[a:b]
        per_core.append((idx_arr, rl_arr, val_arr))
    return quota, per_core


def _build_edge_inputs(edge_row, edge_col, edge_val):
    quota, per_core = _preprocess(edge_row, edge_col, edge_val)
    T = int(quota.sum())
    edge_maps = []
    for c in range(NCORES):
        idx_arr, rl_arr, val_arr = per_core[c]
        calls = []
        for sb in range(NSB):
            o = 0
            for k in range(NCHUNK):
                q = int(quota[k])
                blk = idx_arr[sb * SB:(sb + 1) * SB, o:o + q, :]
                calls.append(blk.reshape(-1).reshape(-1, 16).T)
                o += q
        idx_all = np.tile(np.concatenate(calls, axis=1), (8, 1))
        # Dense one-hot S tiles built on host: S[w, t, e, n] = val (n == rl).
        # Padded slots have val 0. Uploaded instead of building on DVE
        # (frees the vector engine and removes GpSimd SBUF-port contention).
        S4 = np.zeros((NWIN, T, P, P), np.float16)
        np.put_along_axis(S4, rl_arr[..., None], val_arr[..., None], axis=3)
        s_all = np.ascontiguousarray(
            S4.transpose(2, 0, 1, 3).reshape(P, NWIN * T * P))
        edge_maps.append({
            "idx_all": np.ascontiguousarray(idx_all, dtype=np.int16),
            "s_all": s_all,
        })
    return quota, edge_maps


# ------------------------------------------------------------- bass programs

_CACHE = {}


def _bass_mods():
    import concourse.bacc as bacc
    import concourse.tile as tile
    from concourse import mybir
    return bacc, tile, mybir


def _build_prog_a():
    """x1_shard[NPADC, HID] (fp16) = featT.T @ W1 (fp32 accum, fp16 in/out).

    Whole-shard bulk DMAs (2 in, GRP-window out batches) to avoid per-window
    HWDGE issue overhead; PSUM drains alternate scalar/vector engines."""
    bacc, tile, mybir = _bass_mods()
    f32, f16 = mybir.dt.float32, mybir.dt.float16
    AF = mybir.ActivationFunctionType

    nc = bacc.Bacc("TRN2", target_bir_lowering=False, debug=False,
                   num_devices=NCORES)
    featT = nc.dram_tensor("featT", [IN_F, NPADC], f16, kind="ExternalInput")
    W1 = nc.dram_tensor("W1", [IN_F, HID], f16, kind="ExternalInput")
    x1 = nc.dram_tensor("x1", [NPADC, HID], f16, kind="ExternalOutput")
    x1v = x1.rearrange("(w r) c -> w r c", r=P)

    GRP = 14                      # windows per output DMA; 98 = 7 * 14
    with tile.TileContext(nc, num_cores=NCORES) as tc:
        with tc.tile_pool(name="const", bufs=1) as cpool, \
             tc.tile_pool(name="out", bufs=3) as opool, \
             tc.tile_pool(name="ps", bufs=8, space="PSUM") as pspool:
            W1a = cpool.tile([P, HID], f16, tag="W1a")
            nc.sync.dma_start(out=W1a[:], in_=W1[0:P, :])
            W1b = cpool.tile([P, HID], f16, tag="W1b")
            nc.sync.dma_start(out=W1b[:], in_=W1[P:IN_F, :])
            fA = cpool.tile([P, NPADC], f16, tag="fA")
            nc.sync.dma_start(out=fA[:], in_=featT[0:P, :])
            fB = cpool.tile([P, NPADC], f16, tag="fB")
            nc.sync.dma_start(out=fB[:], in_=featT[P:IN_F, :])
            for g in range(NWIN // GRP):
                ot = opool.tile([P, GRP, HID], f16, tag="ot")
                for j in range(GRP):
                    w = g * GRP + j
                    ps = pspool.tile([P, HID], f32, tag="d1")
                    nc.tensor.matmul(ps[:], lhsT=fA[:, w * P:(w + 1) * P],
                                     rhs=W1a[:], start=True, stop=False)
                    nc.tensor.matmul(ps[:], lhsT=fB[:, w * P:(w + 1) * P],
                                     rhs=W1b[:], start=False, stop=True)
                    if j % 2 == 0:
                        nc.scalar.activation(ot[:, j, :], ps[:], AF.Copy)
                    else:
                        nc.vector.tensor_copy(ot[:, j, :], ps[:])
                nc.sync.dma_start(
                    out=x1v[g * GRP:(g + 1) * GRP, :, :].rearrange(
                        "w r c -> r w c"),
                    in_=ot[:])
    nc.compile()
    return nc


def _spmm_phase(nc, tc, mybir, quota, table, layer2, W2t, out, iopool, gpool,
                spool, wpool, pswin, psdense, idx_all, s_all):
    """Emit the spmm super-block loop. layer1: hT = relu(msgs.T @ S) then
    x2 = hT.T @ W2 -> out rows (fp16). layer2: o = softmax(relu(S.T @ msgs))
    -> out rows (fp32). S tiles are host-precomputed and streamed in."""
    f32, f16, i16 = mybir.dt.float32, mybir.dt.float16, mybir.dt.int16
    AF = mybir.ActivationFunctionType
    ALU = mybir.AluOpType

    qs = [int(q) for q in quota]
    T = sum(qs)
    ncall16 = [SB * q * P // 16 for q in qs]
    outw = OUT if layer2 else OUT
    outv = out.rearrange("(w r) c -> w r c", r=P)

    idxcol = 0
    for sb in range(NSB):
        dsts = []
        for k in range(NCHUNK):
            nci = ncall16[k]
            nidx = SB * qs[k] * P
            it = iopool.tile([P, nci], i16, tag=f"idx{k}")
            nc.scalar.dma_start(out=it[:], in_=idx_all[:, idxcol:idxcol + nci])
            dst = gpool.tile([P, SB * qs[k], P], f16, tag=f"gd{k}")
            nc.gpsimd.dma_gather(
                dst[:], table[k * CHROWS:(k + 1) * CHROWS, :],
                it[:], nidx, nidx, P, single_packet=False,
                queue_num=(sb + k) % 4)
            dsts.append(dst)
            idxcol += nci
        ot = wpool.tile([P, SB, outw], f32 if layer2 else f16, tag="ot")
        for wl in range(SB):
            w = sb * SB + wl
            St = spool.tile([P, T, P], f16, tag="St")
            nc.sync.dma_start(
                out=St[:], in_=s_all[:, w * T * P:(w + 1) * T * P])
            acc = pswin.tile([P, P if not layer2 else OUT], f32, tag="acc")
            j = 0
            for k in range(NCHUNK):
                for t in range(qs[k]):
                    if layer2:
                        nc.tensor.matmul(acc[:], lhsT=St[:, j, :],
                                         rhs=dsts[k][:, wl * qs[k] + t, 0:OUT],
                                         start=(j == 0), stop=(j == T - 1))
                    else:
                        nc.tensor.matmul(acc[:], lhsT=dsts[k][:, wl * qs[k] + t, :],
                                         rhs=St[:, j, :],
                                         start=(j == 0), stop=(j == T - 1))
                    j += 1
            if not layer2:
                hT = wpool.tile([P, P], f16, tag="hT")
                nc.scalar.activation(hT[:], acc[:], AF.Relu)
                x2ps = psdense.tile([P, OUT], f32, tag="d2")
                nc.tensor.matmul(x2ps[:], lhsT=hT[:], rhs=W2t[:],
                                 start=True, stop=True)
                nc.scalar.activation(ot[:, wl, :], x2ps[:], AF.Copy)
            else:
                r = wpool.tile([P, OUT], f32, tag="r")
                nc.scalar.activation(r[:], acc[:], AF.Relu)
                nm = wpool.tile([P, 1], f32, tag="nm")
                nc.vector.tensor_reduce(nm[:], r[:],
                                        axis=mybir.AxisListType.X,
                                        op=ALU.max, negate=True)
                ex = wpool.tile([P, OUT], f32, tag="ex")
                se = wpool.tile([P, 1], f32, tag="se")
                nc.scalar.activation(ex[:], r[:], AF.Exp, bias=nm[:],
                                     accum_out=se[:])
                rs = wpool.tile([P, 1], f32, tag="rs")
                nc.vector.reciprocal(rs[:], se[:])
                nc.scalar.activation(ot[:, wl, :], ex[:], AF.Copy, scale=rs[:])
        nc.sync.dma_start(
            out=outv[sb * SB:(sb + 1) * SB, :, :].rearrange("w r c -> r w c"),
            in_=ot[:])


def _build_prog_bc(quota, layer2):
    bacc, tile, mybir = _bass_mods()
    f32, f16, i16 = mybir.dt.float32, mybir.dt.float16, mybir.dt.int16

    qs = [int(q) for q in quota]
    T = sum(qs)
    NIDX = NWIN * T * P // 16

    nc = bacc.Bacc("TRN2", target_bir_lowering=False, debug=False,
                   num_devices=NCORES, num_swdge_queues=4)
    W2 = None
    if layer2:
        table = nc.dram_tensor("x2_full", [NTOT, P], f16,
                               kind="ExternalInput")
        outt = nc.dram_tensor("out", [NPADC, OUT], f32, kind="ExternalOutput")
    else:
        table = nc.dram_tensor("x1_full", [NTOT, HID], f16,
                               kind="ExternalInput")
        outt = nc.dram_tensor("x2", [NPADC, OUT], f16, kind="ExternalOutput")
        W2 = nc.dram_tensor("W2", [HID, OUT], f16, kind="ExternalInput")
    idx_all = nc.dram_tensor("idx_all", [P, NIDX], i16, kind="ExternalInput")
    s_all = nc.dram_tensor("s_all", [P, NWIN * T * P], f16,
                           kind="ExternalInput")

    with tile.TileContext(nc, num_cores=NCORES) as tc:
        with tc.tile_pool(name="const", bufs=1) as cpool, \
             tc.tile_pool(name="io", bufs=4) as iopool, \
             tc.tile_pool(name="gd", bufs=3) as gpool, \
             tc.tile_pool(name="sblk", bufs=8) as spool, \
             tc.tile_pool(name="wout", bufs=3) as wpool, \
             tc.tile_pool(name="psw", bufs=4, space="PSUM") as pswin, \
             tc.tile_pool(name="psd", bufs=2, space="PSUM") as psdense:
            W2t = None
            if not layer2:
                W2t = cpool.tile([P, OUT], f16, tag="W2t")
                nc.sync.dma_start(out=W2t[:], in_=W2[:])
            _spmm_phase(nc, tc, mybir, qs, table, layer2, W2t, outt,
                        iopool, gpool, spool, wpool, pswin, psdense,
                        idx_all, s_all)
    nc.compile()
    return nc


# ------------------------------------------------------------------- kernel

PROFILE = False          # set True (with NTFF hook installed) to trace launches
LAST_PROFILE = []        # [(exec_time_ns, tmpdir), ...] per launch when PROFILE


def _run(prog, maps, cores):
    from concourse.bass_utils import run_bass_kernel_spmd
    kw = {}
    if PROFILE:
        import tempfile
        kw = dict(trace=True, tmpdir=tempfile.mkdtemp(prefix="gnnprof_"))
    r = run_bass_kernel_spmd(prog, maps, cores, **kw)
    if PROFILE:
        LAST_PROFILE.append((r.exec_time_ns, kw.get("tmpdir")))
    return r


def _get_progs(key):
    if key not in _CACHE:
        _CACHE[key] = (_build_prog_a(), _build_prog_bc(key, False),
                       _build_prog_bc(key, True))
    return _CACHE[key]


def kernel(features, edge_row, edge_col, edge_val, W1, W2):
    features = np.asarray(features, dtype=np.float32)
    quota, edge_maps = _build_edge_inputs(
        np.asarray(edge_row, dtype=np.int64),
        np.asarray(edge_col, dtype=np.int64),
        np.asarray(edge_val, dtype=np.float32))
    key = tuple(int(q) for q in quota)
    prog_a, prog_b, prog_c = _get_progs(key)
    cores = list(range(NCORES))
    W1f = np.ascontiguousarray(W1, dtype=np.float16)
    W2f = np.ascontiguousarray(W2, dtype=np.float16)

    # launch A: dense1
    a_maps = []
    for c in range(NCORES):
        f = np.zeros((NPADC, IN_F), np.float16)
        f[:NSHARD] = features[c * NSHARD:(c + 1) * NSHARD].astype(np.float16)
        a_maps.append({"featT": np.ascontiguousarray(f.T), "W1": W1f})
    res_a = _run(prog_a, a_maps, cores)
    x1_full = np.concatenate([res_a.results[c]["x1"] for c in range(NCORES)],
                             axis=0)

    # launch B: spmm1 + dense2
    b_maps = [{"x1_full": x1_full, "W2": W2f, **edge_maps[c]}
              for c in range(NCORES)]
    res_b = _run(prog_b, b_maps, cores)
    x2_full = np.zeros((NTOT, P), np.float16)
    x2_full[:, :OUT] = np.concatenate(
        [res_b.results[c]["x2"] for c in range(NCORES)], axis=0)

    # launch C: spmm2 + softmax
    c_maps = [{"x2_full": x2_full, **edge_maps[c]} for c in range(NCORES)]
    res_c = _run(prog_c, c_maps, cores)
    return np.concatenate(
        [res_c.results[c]["out"][:NSHARD] for c in range(NCORES)],
        axis=0).astype(np.float32)

